# revision 38
# baseline (speedup 1.0000x reference)
"""Trainium2 Bass kernel for the controlled-unitary problem.

reference semantics (control=0, num_qubits=13, dim=8192):
    mask bit = 1 << 12, so columns/rows with that bit set are idx 4096..8191.
    out[:, c0] = state[:, c0]                       (control bit off: untouched)
    out[:, c1] = state[:, c1] @ target[c1, c1]      (controlled unitary)

Device work: complex [256,4096] @ [4096,4096] GEMM = 3 real GEMMs (Gauss).
Sharding: output columns of the GEMM split 8 ways (each core gets a
[4096, 512] slab of the target block; every weight byte moves once).

Per-core kernel (final):
  Gauss 3-mult complex GEMM with plane-combines pushed to the host:
      m1 = a_r  . (b_r + b_i)        (bs = b_r + b_i made on-chip, DVE)
      m2 = (a_r + a_i) . b_i         (aps host-precomputed)
      m3 = (a_i - a_r) . b_r         (ams host-precomputed)
      C_r = m1 - m2,  C_i = m1 + m3
  fp16 operands (rel err ~3e-4, tolerance 2e-2); fp16 outputs.

  What made it fast (78.5us -> ~64.5us; fp16 PE floor is ~43us + ~13us
  of fixed NEFF preamble/teardown + ramp):
  - Everything SBUF-resident (~15MB): input DMA never waits on PE
    consumption (no tile-pool rotation), so it free-runs ahead.
  - Merged per-ring DRAM layouts (d1 = [aps|br|ams] on the SP ring,
    d2 = [bi|ar] on the ACT ring) -> one dma_start per chunk per ring
    with k-chunk-contiguous runs (2KB per ktile per partition), which
    keeps both HWDGE queues at their ~200-250GB/s (aggregate ~420GB/s).
  - Chunk-end gating model: a chunk's first matmul waits for the whole
    chunk DMA, so chunk size is bounded by supply rate (1.13us/ktile)
    vs PE demand (1.34us/ktile): stall ~ 1.13*ch - 0.21*K_start - 1.1.
    Hence ramped [1,1,2,4] head and all-4 tail (8-ktile chunks stall).
  - PE warmup matmuls on a zeroed tile keep the p-state ramp running
    during the DMA head.
  - Within each chunk PE order is m2, m3, m1 so the DVE bs add has slack.
  - Last chunk is product-major (m1, m3, m2) with per-m-tile combines:
    only m1 needs a PSUM->SBUF copy (Act engine); C_i = m3 + t1 and
    -C_r = m2 - t1 read m2/m3 straight from PSUM (one-PSUM-operand rule),
    with the sign flip absorbed on the host.
"""

import numpy as np

BATCH = 256
DIM = 8192
HALF = 4096
N_CORES = 8
NSH = HALF // N_CORES  # 512 output columns per core
KT = HALF // 128  # 32 k-tiles
MT = BATCH // 128  # 2 m-tiles
CHUNKS = [1, 1, 1, 1, 2, 2, 2, 3, 3, 4, 4, 4, 4]  # k-tiles per chunk (sum KT)
CHMAX = max(CHUNKS)
WARM = 3  # p-state warmup matmuls before real work

DT_NAME = "float16"

_CACHE = {}


def _build():
    import concourse.mybir as mybir
    import concourse.tile as tile
    from concourse import bacc

    DT = mybir.dt.float16
    F32 = mybir.dt.float32

    nc = bacc.Bacc("TRN2", target_bir_lowering=False, debug=False,
                   num_devices=N_CORES)

    # merged per-ring input layouts: one contiguous run per (partition,
    # k-chunk) so every DMA moves >=1.75KB descriptors even for 1-ktile
    # chunks. Rings are BALANCED at 896 fp16/ktile/partition each (the
    # 256-wide ams plane splits into its two 128-wide m-tile slabs):
    #   d1 (sync ring):   [aps | bi | ams_m0] -> 896 fp16 per ktile
    #   d2 (scalar ring): [br | ams_m1 | ar]  -> 896 fp16 per ktile
    RING = BATCH + NSH + 128  # 896
    d1 = nc.dram_tensor("d1", [128, KT, RING], DT, kind="ExternalInput")
    d2 = nc.dram_tensor("d2", [128, KT, RING], DT, kind="ExternalInput")
    c_r = nc.dram_tensor("c_r", [BATCH, NSH], DT, kind="ExternalOutput")
    c_i = nc.dram_tensor("c_i", [BATCH, NSH], DT, kind="ExternalOutput")

    with tile.TileContext(nc) as tc:
        with (
            tc.tile_pool(name="ap", bufs=1) as ap_pool,
            tc.tile_pool(name="bp", bufs=1) as bp_pool,
            tc.tile_pool(name="op", bufs=2) as o_pool,
            tc.tile_pool(name="wp", bufs=1) as w_pool,
            tc.tile_pool(name="ps", bufs=1, space="PSUM") as ps_pool,
        ):
            ps = {}
            for m in range(MT):
                for comp in ("m1", "m2", "m3"):
                    ps[(m, comp)] = ps_pool.tile(
                        [128, NSH], F32, name=f"ps_{m}_{comp}"
                    )
            ps_warm = ps_pool.tile([128, NSH], F32, name="ps_warm")

            # Everything SBUF-resident (~15MB of 24MB): input DMA never
            # waits on PE consumption, so it free-runs ahead of compute.
            d1_t = ap_pool.tile([128, KT, RING], DT, name="d1f")
            d2_t = ap_pool.tile([128, KT, RING], DT, name="d2f")
            bs_t = bp_pool.tile([128, KT, NSH], DT, name="bsf")

            # sub-views of the merged layouts
            APS, BIS, AMS0 = 0, BATCH, BATCH + NSH   # in d1
            BRS, AMS1, ARS = 0, NSH, NSH + 128       # in d2

            # p-state warmup: keep the PE busy on zeros while input DMA
            # streams in, so the clock is ramped when real matmuls start.
            # memset on GpSimd (fast + otherwise idle) so DVE stays free.
            wtile = w_pool.tile([128, NSH], DT, name="warm")
            nc.gpsimd.memset(wtile[:], 0.0)
            for _ in range(WARM):
                nc.tensor.matmul(ps_warm[:], wtile[:, :128], wtile[:],
                                 start=True, stop=True)

            def issue(comp, m, k, stop):
                if comp == "m1":
                    lhs = d2_t[:, k, ARS + m * 128:ARS + (m + 1) * 128]
                    rhs = bs_t[:, k, :]
                elif comp == "m2":
                    lhs = d1_t[:, k, APS + m * 128:APS + (m + 1) * 128]
                    rhs = d1_t[:, k, BIS:BIS + NSH]
                else:  # m3 lhs: ams m-slab 0 lives on d1, slab 1 on d2
                    lhs = (d1_t[:, k, AMS0:AMS0 + 128] if m == 0
                           else d2_t[:, k, AMS1:AMS1 + 128])
                    rhs = d2_t[:, k, BRS:BRS + NSH]
                nc.tensor.matmul(
                    ps[(m, comp)][:], lhs, rhs, start=(k == 0), stop=stop,
                )

            k0 = 0
            for ci, ch in enumerate(CHUNKS):
                ksl = slice(k0, k0 + ch)
                if ci == 0:
                    # split ktile 0 so m2's operands (aps, bi) land first
                    # on d1 and m3's (br, ams1) first on d2
                    nc.sync.dma_start(d1_t[:, ksl, APS:AMS0],
                                      d1[:, ksl, APS:AMS0])
                    nc.sync.dma_start(d1_t[:, ksl, AMS0:],
                                      d1[:, ksl, AMS0:])
                    nc.scalar.dma_start(d2_t[:, ksl, BRS:ARS],
                                        d2[:, ksl, BRS:ARS])
                    nc.scalar.dma_start(d2_t[:, ksl, ARS:],
                                        d2[:, ksl, ARS:])
                else:
                    nc.sync.dma_start(d1_t[:, ksl, :], d1[:, ksl, :])
                    nc.scalar.dma_start(d2_t[:, ksl, :], d2[:, ksl, :])
                # the only on-chip operand prep: bs = b_r + b_i
                nc.vector.tensor_tensor(bs_t[:, ksl, :],
                                        d2_t[:, ksl, BRS:BRS + NSH],
                                        d1_t[:, ksl, BIS:BIS + NSH],
                                        mybir.AluOpType.add)
                last_chunk = k0 + ch == KT

                if not last_chunk:
                    # product-major: m2/m3 need only DMA'd planes, m1 last
                    # so the DVE has slack to produce bs.
                    for comp in ("m2", "m3", "m1"):
                        for m in range(MT):
                            for kk in range(ch):
                                issue(comp, m, k0 + kk, False)
                else:
                    # Last chunk: product-major (m1, m3, m2), with combines
                    # fired per m-tile as soon as their banks close. Only m1
                    # needs a PSUM->SBUF staging copy (t1, Act engine); the
                    # vector ops read m3/m2 straight from PSUM:
                    #   out_i  = m3 + t1 =  C_i
                    #   out_rn = m2 - t1 = -C_r   (negated back on the host)
                    t1s, outs = {}, {}
                    for m in range(MT):
                        outs[m] = (
                            o_pool.tile([128, NSH], DT, name=f"out_r{m}"),
                            o_pool.tile([128, NSH], DT, name=f"out_i{m}"),
                        )
                        t1s[m] = o_pool.tile([128, NSH], F32, name=f"t1_{m}")
                    for m in range(MT):
                        for kk in range(ch):
                            issue("m1", m, k0 + kk, kk == ch - 1)
                        # PSUM->SBUF staging on DVE, NOT Act: any use of
                        # nc.scalar.activation makes the compiler emit a
                        # 1.3us ACT_TABLE_LOAD on Scalar during the
                        # preamble, delaying that ring's first input DMA.
                        nc.vector.tensor_scalar_add(
                            t1s[m][:], ps[(m, "m1")][:], 0.0)
                    for m in range(MT):
                        for kk in range(ch):
                            issue("m3", m, k0 + kk, kk == ch - 1)
                        nc.vector.tensor_tensor(
                            outs[m][1][:], ps[(m, "m3")][:], t1s[m][:],
                            mybir.AluOpType.add)
                        nc.scalar.dma_start(
                            c_i[m * 128:(m + 1) * 128, :], outs[m][1][:])
                    for m in range(MT):
                        for kk in range(ch):
                            issue("m2", m, k0 + kk, kk == ch - 1)
                        nc.vector.tensor_tensor(
                            outs[m][0][:], ps[(m, "m2")][:], t1s[m][:],
                            mybir.AluOpType.subtract)
                        nc.sync.dma_start(
                            c_r[m * 128:(m + 1) * 128, :], outs[m][0][:])
                k0 += ch

    nc.compile()
    return nc


def _get_nc():
    if "nc" not in _CACHE:
        _CACHE["nc"] = _build()
    return _CACHE["nc"]


def _pack_kxm(mat_t):
    # mat_t: [4096, F] (k-major) -> [128, KT, F] with k = kt*128 + p
    f = mat_t.shape[1]
    return np.ascontiguousarray(
        mat_t.reshape(KT, 128, f).transpose(1, 0, 2).astype(np.float16)
    )


def run_device(A, B, dt_name=DT_NAME, trace=False):
    """A: [256, 4096] complex64, B: [4096, 4096] complex64.
    Returns C = A @ B as [256, 4096] complex64 plus the raw results."""
    from concourse import bass_utils

    nc = _get_nc()

    at = A.T  # [4096, 256]
    ar = np.ascontiguousarray(at.real).astype(np.float32)
    ai = np.ascontiguousarray(at.imag).astype(np.float32)
    a_r = _pack_kxm(ar)
    a_ps = _pack_kxm(ar + ai)
    a_ms = _pack_kxm(ai - ar)
    br_full = B.real
    bi_full = B.imag

    in_maps = []
    for c in range(N_CORES):
        csl = slice(c * NSH, (c + 1) * NSH)
        b_r = _pack_kxm(np.ascontiguousarray(br_full[:, csl]))
        b_i = _pack_kxm(np.ascontiguousarray(bi_full[:, csl]))
        # balanced per-ring layouts (see _build):
        #   d1 = [aps|bi|ams_m0], d2 = [br|ams_m1|ar]  (896 each)
        d1 = np.concatenate([a_ps, b_i, a_ms[:, :, :128]], axis=2)
        d2 = np.concatenate([b_r, a_ms[:, :, 128:], a_r], axis=2)
        in_maps.append({
            "d1": np.ascontiguousarray(d1),
            "d2": np.ascontiguousarray(d2),
        })

    res = bass_utils.run_bass_kernel_spmd(
        nc, in_maps, core_ids=list(range(N_CORES)), trace=trace
    )

    out = np.empty((BATCH, HALF), dtype=np.complex64)
    for c in range(N_CORES):
        csl = slice(c * NSH, (c + 1) * NSH)
        # device returns c_r negated (m2 - m1); flip sign here for free
        out.real[:, csl] = -res.results[c]["c_r"].astype(np.float32)
        out.imag[:, csl] = res.results[c]["c_i"].astype(np.float32)
    return out, res


def kernel(state, target_matrix, control, num_qubits):
    state = np.asarray(state)
    target_matrix = np.asarray(target_matrix)
    control = int(control)
    num_qubits = int(num_qubits)
    dim = 1 << num_qubits

    assert state.shape == (BATCH, DIM) and dim == DIM, (
        "kernel hardcoded for [256, 8192]"
    )

    mask = 1 << (num_qubits - control - 1)
    idx = np.arange(dim)
    c1 = idx[(idx & mask) != 0]  # columns with control bit set

    if control == 0:
        A = state[:, HALF:]
        B = target_matrix[HALF:, HALF:]
    else:
        A = state[:, c1]
        B = target_matrix[np.ix_(c1, c1)]
    A = np.ascontiguousarray(A, dtype=np.complex64)
    B = np.ascontiguousarray(B, dtype=np.complex64)

    C, _ = run_device(A, B)

    out = state.astype(np.complex64, copy=True)
    out[:, c1] = C
    return out


# revision 40
# speedup vs baseline: 1.0272x; 1.0272x over previous
"""Trainium2 Bass kernel for the controlled-unitary problem.

reference semantics (control=0, num_qubits=13, dim=8192):
    mask bit = 1 << 12, so columns/rows with that bit set are idx 4096..8191.
    out[:, c0] = state[:, c0]                       (control bit off: untouched)
    out[:, c1] = state[:, c1] @ target[c1, c1]      (controlled unitary)

Device work: complex [256,4096] @ [4096,4096] GEMM = 3 real GEMMs (Gauss).
Sharding: output columns of the GEMM split 8 ways (each core gets a
[4096, 512] slab of the target block; every weight byte moves once).

Per-core kernel (final):
  Gauss 3-mult complex GEMM with plane-combines pushed to the host:
      m1 = a_r  . (b_r + b_i)        (bs = b_r + b_i made on-chip, DVE)
      m2 = (a_r + a_i) . b_i         (aps host-precomputed)
      m3 = (a_i - a_r) . b_r         (ams host-precomputed)
      C_r = m1 - m2,  C_i = m1 + m3
  fp16 operands (rel err ~3e-4, tolerance 2e-2); fp16 outputs.

  What made it fast (78.5us -> ~64.5us; fp16 PE floor is ~43us + ~13us
  of fixed NEFF preamble/teardown + ramp):
  - Everything SBUF-resident (~15MB): input DMA never waits on PE
    consumption (no tile-pool rotation), so it free-runs ahead.
  - Merged per-ring DRAM layouts (d1 = [aps|br|ams] on the SP ring,
    d2 = [bi|ar] on the ACT ring) -> one dma_start per chunk per ring
    with k-chunk-contiguous runs (2KB per ktile per partition), which
    keeps both HWDGE queues at their ~200-250GB/s (aggregate ~420GB/s).
  - Chunk-end gating model: a chunk's first matmul waits for the whole
    chunk DMA, so chunk size is bounded by supply rate (1.13us/ktile)
    vs PE demand (1.34us/ktile): stall ~ 1.13*ch - 0.21*K_start - 1.1.
    Hence ramped [1,1,2,4] head and all-4 tail (8-ktile chunks stall).
  - PE warmup matmuls on a zeroed tile keep the p-state ramp running
    during the DMA head.
  - Within each chunk PE order is m2, m3, m1 so the DVE bs add has slack.
  - Last chunk is product-major (m1, m3, m2) with per-m-tile combines:
    only m1 needs a PSUM->SBUF copy (Act engine); C_i = m3 + t1 and
    -C_r = m2 - t1 read m2/m3 straight from PSUM (one-PSUM-operand rule),
    with the sign flip absorbed on the host.
"""

import numpy as np

BATCH = 256
DIM = 8192
HALF = 4096
N_CORES = 8
NSH = HALF // N_CORES  # 512 output columns per core
KT = HALF // 128  # 32 k-tiles
MT = BATCH // 128  # 2 m-tiles
CHUNKS = [1, 1, 2, 4, 4, 4, 4, 4, 4, 4]  # k-tiles per DMA chunk (sums to KT)
CHMAX = max(CHUNKS)
WARM = 5  # p-state warmup matmuls: fill PE idle until chunk0 DMA lands
          # (~10.9us; DMA rings can't start before ~8.7us post-barrier)

DT_NAME = "float16"

_CACHE = {}


def _build():
    import concourse.mybir as mybir
    import concourse.tile as tile
    from concourse import bacc

    DT = mybir.dt.float16
    F32 = mybir.dt.float32

    nc = bacc.Bacc("TRN2", target_bir_lowering=False, debug=False,
                   num_devices=N_CORES)

    # merged per-ring input layouts: one contiguous run per (partition,
    # k-chunk) so every DMA moves >=1.75KB descriptors even for 1-ktile
    # chunks. Rings are BALANCED at 896 fp16/ktile/partition each (the
    # 256-wide ams plane splits into its two 128-wide m-tile slabs):
    #   d1 (sync ring):   [aps | bi | ams_m0] -> 896 fp16 per ktile
    #   d2 (scalar ring): [br | ams_m1 | ar]  -> 896 fp16 per ktile
    RING = BATCH + NSH + 128  # 896
    d1 = nc.dram_tensor("d1", [128, KT, RING], DT, kind="ExternalInput")
    d2 = nc.dram_tensor("d2", [128, KT, RING], DT, kind="ExternalInput")
    c_r = nc.dram_tensor("c_r", [BATCH, NSH], DT, kind="ExternalOutput")
    c_i = nc.dram_tensor("c_i", [BATCH, NSH], DT, kind="ExternalOutput")

    with tile.TileContext(nc) as tc:
        with (
            tc.tile_pool(name="ap", bufs=1) as ap_pool,
            tc.tile_pool(name="bp", bufs=1) as bp_pool,
            tc.tile_pool(name="op", bufs=2) as o_pool,
            tc.tile_pool(name="wp", bufs=1) as w_pool,
            tc.tile_pool(name="ps", bufs=1, space="PSUM") as ps_pool,
        ):
            ps = {}
            for m in range(MT):
                for comp in ("m1", "m2", "m3"):
                    ps[(m, comp)] = ps_pool.tile(
                        [128, NSH], F32, name=f"ps_{m}_{comp}"
                    )
            ps_warm = ps_pool.tile([128, NSH], F32, name="ps_warm")

            # Everything SBUF-resident (~15MB of 24MB): input DMA never
            # waits on PE consumption, so it free-runs ahead of compute.
            d1_t = ap_pool.tile([128, KT, RING], DT, name="d1f")
            d2_t = ap_pool.tile([128, KT, RING], DT, name="d2f")
            bs_t = bp_pool.tile([128, KT, NSH], DT, name="bsf")

            # sub-views of the merged layouts
            APS, BIS, AMS0 = 0, BATCH, BATCH + NSH   # in d1
            BRS, AMS1, ARS = 0, NSH, NSH + 128       # in d2

            # p-state warmup: keep the PE busy on zeros while input DMA
            # streams in, so the clock is ramped when real matmuls start.
            # memset on GpSimd (fast + otherwise idle) so DVE stays free.
            wtile = w_pool.tile([128, NSH], DT, name="warm")
            nc.gpsimd.memset(wtile[:], 0.0)
            for _ in range(WARM):
                nc.tensor.matmul(ps_warm[:], wtile[:, :128], wtile[:],
                                 start=True, stop=True)

            def issue(comp, m, k, stop):
                if comp == "m1":
                    lhs = d2_t[:, k, ARS + m * 128:ARS + (m + 1) * 128]
                    rhs = bs_t[:, k, :]
                elif comp == "m2":
                    lhs = d1_t[:, k, APS + m * 128:APS + (m + 1) * 128]
                    rhs = d1_t[:, k, BIS:BIS + NSH]
                else:  # m3 lhs: ams m-slab 0 lives on d1, slab 1 on d2
                    lhs = (d1_t[:, k, AMS0:AMS0 + 128] if m == 0
                           else d2_t[:, k, AMS1:AMS1 + 128])
                    rhs = d2_t[:, k, BRS:BRS + NSH]
                nc.tensor.matmul(
                    ps[(m, comp)][:], lhs, rhs, start=(k == 0), stop=stop,
                )

            k0 = 0
            for ci, ch in enumerate(CHUNKS):
                ksl = slice(k0, k0 + ch)
                if ci == 0:
                    # split ktile 0 so m2's operands (aps, bi) land first
                    # on d1 and m3's (br, ams1) first on d2
                    nc.sync.dma_start(d1_t[:, ksl, APS:AMS0],
                                      d1[:, ksl, APS:AMS0])
                    nc.sync.dma_start(d1_t[:, ksl, AMS0:],
                                      d1[:, ksl, AMS0:])
                    nc.scalar.dma_start(d2_t[:, ksl, BRS:ARS],
                                        d2[:, ksl, BRS:ARS])
                    nc.scalar.dma_start(d2_t[:, ksl, ARS:],
                                        d2[:, ksl, ARS:])
                else:
                    nc.sync.dma_start(d1_t[:, ksl, :], d1[:, ksl, :])
                    nc.scalar.dma_start(d2_t[:, ksl, :], d2[:, ksl, :])
                # the only on-chip operand prep: bs = b_r + b_i
                nc.vector.tensor_tensor(bs_t[:, ksl, :],
                                        d2_t[:, ksl, BRS:BRS + NSH],
                                        d1_t[:, ksl, BIS:BIS + NSH],
                                        mybir.AluOpType.add)
                last_chunk = k0 + ch == KT

                if not last_chunk:
                    # product-major: m2/m3 need only DMA'd planes, m1 last
                    # so the DVE has slack to produce bs.
                    for comp in ("m2", "m3", "m1"):
                        for m in range(MT):
                            for kk in range(ch):
                                issue(comp, m, k0 + kk, False)
                else:
                    # Last chunk: product-major (m1, m3, m2), with combines
                    # fired per m-tile as soon as their banks close. Only m1
                    # needs a PSUM->SBUF staging copy (t1, Act engine); the
                    # vector ops read m3/m2 straight from PSUM:
                    #   out_i  = m3 + t1 =  C_i
                    #   out_rn = m2 - t1 = -C_r   (negated back on the host)
                    t1s, outs = {}, {}
                    for m in range(MT):
                        outs[m] = (
                            o_pool.tile([128, NSH], DT, name=f"out_r{m}"),
                            o_pool.tile([128, NSH], DT, name=f"out_i{m}"),
                        )
                        t1s[m] = o_pool.tile([128, NSH], F32, name=f"t1_{m}")
                    for m in range(MT):
                        for kk in range(ch):
                            issue("m1", m, k0 + kk, kk == ch - 1)
                        # PSUM->SBUF staging on DVE, NOT Act: any use of
                        # nc.scalar.activation makes the compiler emit a
                        # 1.3us ACT_TABLE_LOAD on Scalar during the
                        # preamble, delaying that ring's first input DMA.
                        nc.vector.tensor_scalar_add(
                            t1s[m][:], ps[(m, "m1")][:], 0.0)
                    for m in range(MT):
                        for kk in range(ch):
                            issue("m3", m, k0 + kk, kk == ch - 1)
                        nc.vector.tensor_tensor(
                            outs[m][1][:], ps[(m, "m3")][:], t1s[m][:],
                            mybir.AluOpType.add)
                        nc.scalar.dma_start(
                            c_i[m * 128:(m + 1) * 128, :], outs[m][1][:])
                    for m in range(MT):
                        for kk in range(ch):
                            issue("m2", m, k0 + kk, kk == ch - 1)
                        nc.vector.tensor_tensor(
                            outs[m][0][:], ps[(m, "m2")][:], t1s[m][:],
                            mybir.AluOpType.subtract)
                        nc.sync.dma_start(
                            c_r[m * 128:(m + 1) * 128, :], outs[m][0][:])
                k0 += ch

    nc.compile()
    return nc


def _get_nc():
    if "nc" not in _CACHE:
        _CACHE["nc"] = _build()
    return _CACHE["nc"]


def _pack_kxm(mat_t):
    # mat_t: [4096, F] (k-major) -> [128, KT, F] with k = kt*128 + p
    f = mat_t.shape[1]
    return np.ascontiguousarray(
        mat_t.reshape(KT, 128, f).transpose(1, 0, 2).astype(np.float16)
    )


def run_device(A, B, dt_name=DT_NAME, trace=False):
    """A: [256, 4096] complex64, B: [4096, 4096] complex64.
    Returns C = A @ B as [256, 4096] complex64 plus the raw results."""
    from concourse import bass_utils

    nc = _get_nc()

    at = A.T  # [4096, 256]
    ar = np.ascontiguousarray(at.real).astype(np.float32)
    ai = np.ascontiguousarray(at.imag).astype(np.float32)
    a_r = _pack_kxm(ar)
    a_ps = _pack_kxm(ar + ai)
    a_ms = _pack_kxm(ai - ar)
    br_full = B.real
    bi_full = B.imag

    in_maps = []
    for c in range(N_CORES):
        csl = slice(c * NSH, (c + 1) * NSH)
        b_r = _pack_kxm(np.ascontiguousarray(br_full[:, csl]))
        b_i = _pack_kxm(np.ascontiguousarray(bi_full[:, csl]))
        # balanced per-ring layouts (see _build):
        #   d1 = [aps|bi|ams_m0], d2 = [br|ams_m1|ar]  (896 each)
        d1 = np.concatenate([a_ps, b_i, a_ms[:, :, :128]], axis=2)
        d2 = np.concatenate([b_r, a_ms[:, :, 128:], a_r], axis=2)
        in_maps.append({
            "d1": np.ascontiguousarray(d1),
            "d2": np.ascontiguousarray(d2),
        })

    res = bass_utils.run_bass_kernel_spmd(
        nc, in_maps, core_ids=list(range(N_CORES)), trace=trace
    )

    out = np.empty((BATCH, HALF), dtype=np.complex64)
    for c in range(N_CORES):
        csl = slice(c * NSH, (c + 1) * NSH)
        # device returns c_r negated (m2 - m1); flip sign here for free
        out.real[:, csl] = -res.results[c]["c_r"].astype(np.float32)
        out.imag[:, csl] = res.results[c]["c_i"].astype(np.float32)
    return out, res


def kernel(state, target_matrix, control, num_qubits):
    state = np.asarray(state)
    target_matrix = np.asarray(target_matrix)
    control = int(control)
    num_qubits = int(num_qubits)
    dim = 1 << num_qubits

    assert state.shape == (BATCH, DIM) and dim == DIM, (
        "kernel hardcoded for [256, 8192]"
    )

    mask = 1 << (num_qubits - control - 1)
    idx = np.arange(dim)
    c1 = idx[(idx & mask) != 0]  # columns with control bit set

    if control == 0:
        A = state[:, HALF:]
        B = target_matrix[HALF:, HALF:]
    else:
        A = state[:, c1]
        B = target_matrix[np.ix_(c1, c1)]
    A = np.ascontiguousarray(A, dtype=np.complex64)
    B = np.ascontiguousarray(B, dtype=np.complex64)

    C, _ = run_device(A, B)

    out = state.astype(np.complex64, copy=True)
    out[:, c1] = C
    return out
